# revision 16
# baseline (speedup 1.0000x reference)
"""Trainium2 Bass kernel for the GRU decoder (nn_Decoder_13168369730058).

Math (from the reference):
  h0 = encoder_outputs[0, :, -1, :]                       # (128, 512)
  step 1:   h1 = gru_cell(x=0, h0)
  step t>1: h_t = gru_cell(h_{t-1}, h_{t-1})   (carry is (h_new, h_new))

Because x == h from step 2 on, the two GRU matmuls fuse per gate:
  r  = sigmoid(Wr h + br)          Wr = Wih_r + Whh_r,  br = bih_r + bhh_r
  z  = sigmoid(Wz h + bz)          Wz = Wih_z + Whh_z,  bz = bih_z + bhh_z
  n  = tanh(Win h + bin + r * (Whn h + bhn))
  h' = n + z * (h - n)
Step 1 is the same with Wr,Wz -> Whh_{r,z} and no Win matmul (x = 0).

Distribution: data-parallel over batch, 16 rows per core on 8 cores, weights
replicated; the out_len recurrence is local to each core.

Layout: fully transposed on chip (H on partitions, batch on free dim). Each
128x128 fp16 weight block is the stationary operand (fast-weight-load;
LDW+MM pairs sustain ~27ns in a dense stream) and the transposed hidden
state h^T (128, 16) the moving operand, producing gate pre-activations in
PSUM directly. fp8 weights were measured SLOWER per pair (30ns) - FWL on
this silicon is already at its 32-bit/partition/cycle ceiling with fp16.

Step schedule (PE order):
  [bias_r][r x16][bias_hn][hn x16][bias_in][in x16][idMM][bias_za]
  [bias_zb][z_a x8][z_b x8]
  - biases are seeded into each gate's PSUM by matmuls whose stationaries
    are zero-padded to 128 partitions: a 4-partition stationary disables
    fast-weight-load and stalls the PE pipeline ~200ns per load;
  - r*hn is injected into the in-gate PSUM by an identity matmul so tanh
    reads the complete pre-activation from PSUM (no DVE add);
  - the z-gate runs LAST, split into two PSUM tiles (separate banks) so
    sigmoid_za overlaps the z_b matmuls and the final h' is produced in
    two halves, releasing the next step's k0/k1 matmuls early.
add_dep_helper pins the z-blocks after the idMM and the z-sigmoids after
tanh (the scheduler otherwise serializes the tail). State, tail tensors,
and the DMA'd output are all fp16 (measured 6.2e-4 rel err end-to-end;
the trajectory is contractive, not chaotic).
"""

import os
import numpy as np

import concourse.bacc as bacc
import concourse.mybir as mybir
import concourse.tile as tile
from concourse.tile import add_dep_helper
from concourse.bass_utils import run_bass_kernel_spmd

H = 512
BATCH = 128
N_CORES = int(os.environ.get("GRU_N_CORES", "8"))
T_STEPS = int(os.environ.get("GRU_T_STEPS", "1024"))
B_LOC = BATCH // N_CORES  # local batch per core (16)
KT = H // 128             # 4 k-tiles

F32 = mybir.dt.float32
F16 = mybir.dt.float16


def _build(T: int, b: int):
    """Build the Bass program: T steps, b batch rows per core."""
    nc = bacc.Bacc()

    # wc blocks: [r x16, hn x16, in x16, z x16]; each gate m-tile-major, then k
    wc_d = nc.dram_tensor("wc", [128, 64 * 128], F16, kind="ExternalInput")
    # w1 blocks: [r x16, hn x16, z x16] (step 1, weights = W_hh only)
    w1_d = nc.dram_tensor("w1", [128, 48 * 128], F16, kind="ExternalInput")
    # bias stationaries, zero-padded to 128 partitions: blocks [r|hn|in|z],
    # rows 0-3 of each block hold bias[128k : 128(k+1)] for m-tile k
    bst_d = nc.dram_tensor("bst", [128, 4 * 128], F16, kind="ExternalInput")
    # ones: rows 0-3 carry the m-tile selector pattern, rows 4-127 zero
    ones_d = nc.dram_tensor("ones", [128, 4 * b], F16, kind="ExternalInput")
    ident_d = nc.dram_tensor("ident", [128, 128], F16, kind="ExternalInput")
    h0_d = nc.dram_tensor("h0t", [128, 4 * b], F16, kind="ExternalInput")
    out_d = nc.dram_tensor("outT", [T, 128, 4 * b], F16, kind="ExternalOutput")

    sig = mybir.ActivationFunctionType.Sigmoid
    tanh = mybir.ActivationFunctionType.Tanh

    with tile.TileContext(nc) as tc:
        with (
            tc.tile_pool(name="singles", bufs=1) as singles,
            tc.tile_pool(name="state", bufs=2) as state,
            tc.tile_pool(name="work", bufs=2) as work,
            tc.tile_pool(name="psum", bufs=2, space="PSUM") as psum,
            tc.tile_pool(name="psumz", bufs=1, space="PSUM") as psumz,
        ):
            w_sb = singles.tile([128, 64 * 128], F16)
            nc.sync.dma_start(w_sb[:], wc_d[:])
            w1_sb = singles.tile([128, 48 * 128], F16)
            nc.sync.dma_start(w1_sb[:], w1_d[:])
            bias_sb = {}
            for gi, gname in enumerate(("r", "hn", "in", "z")):
                t_ = singles.tile([128, 128], F16, tag=f"b{gname}")
                nc.sync.dma_start(t_[:], bst_d[:, 128 * gi : 128 * (gi + 1)])
                bias_sb[gname] = t_
            ones_sb = singles.tile([128, 4 * b], F16)
            nc.sync.dma_start(ones_sb[:], ones_d[:])
            ident_sb = singles.tile([128, 128], F16)
            nc.sync.dma_start(ident_sb[:], ident_d[:])

            h16 = state.tile([128, 4 * b], F16, tag="h16")
            nc.sync.dma_start(h16[:], h0_d[:])

            # Warm-up: hardware allows ONE embedded sync wait per instruction;
            # have the PE observe every init DMA here so loop matmuls carry a
            # single cross-engine wait (on h16 only).
            warm_ps = psum.tile([128, 8], F32, tag="r")
            nc.tensor.matmul(warm_ps[:, 0:8], w_sb[:, 0:128], w_sb[:, 0:8],
                             start=True, stop=True)
            nc.tensor.matmul(warm_ps[:, 0:8], w1_sb[:, 0:128], w1_sb[:, 0:8],
                             start=True, stop=True)
            nc.tensor.matmul(warm_ps[:, 0:8], ident_sb[:], w_sb[:, 0:8],
                             start=True, stop=True)
            for gname in ("r", "hn", "in", "z"):
                nc.tensor.matmul(warm_ps[:, 0:8], bias_sb[gname][:],
                                 ones_sb[:, 0:8], start=True, stop=True)

            for t in range(T):
                first = t == 0
                w = w1_sb if first else w_sb

                r_ps = psum.tile([128, 4 * b], F32, tag="r")
                hn_ps = psum.tile([128, 4 * b], F32, tag="hn")
                in_ps = psum.tile([128, 4 * b], F32, tag="in")
                za_ps = psumz.tile([128, 2 * b], F32, tag="za")
                zb_ps = psumz.tile([128, 2 * b], F32, tag="zb")

                # The whole PE queue is order-pinned (order-only deps): the
                # scheduler otherwise interleaves blocks in ways that push
                # serial-chain producers (r-block, idMM) late.
                prev_mm = [None]

                def chain(mm):
                    if prev_mm[0] is not None:
                        add_dep_helper(mm.ins, prev_mm[0].ins, sync=False,
                                       reason="PE queue order")
                    prev_mm[0] = mm
                    return mm

                def bias_mm(ps, gname, cols=None, stop=False):
                    rhs = ones_sb[:] if cols is None else ones_sb[:, cols]
                    chain(nc.tensor.matmul(ps, bias_sb[gname][:], rhs,
                                           start=True, stop=stop,
                                           skip_group_check=True))

                def mm_k(ps, g_idx, ks, tiles=(0, 1, 2, 3), col_of=None,
                         stop_at=None):
                    for k in ks:
                        for ti, tt in enumerate(tiles):
                            blk = ((g_idx * 4 + tt) * KT + k) * 128
                            col = tt if col_of is None else col_of[ti]
                            chain(nc.tensor.matmul(
                                ps[:, col * b : (col + 1) * b],
                                w[:, blk : blk + 128],
                                h16[:, k * b : (k + 1) * b],
                                start=False,
                                stop=(stop_at == (k, tt)),
                                skip_group_check=True,
                            ))

                zg = 2 if first else 3
                # k0/k1 matmuls depend only on the early half of h'; the
                # r-block's k2/k3 go first after h'_b so sigma_r fires ASAP.
                bias_mm(r_ps[:], "r")
                mm_k(r_ps, 0, (0, 1))
                bias_mm(in_ps[:], "in", stop=first)
                if not first:
                    mm_k(in_ps, 2, (0, 1))
                mm_k(r_ps, 0, (2, 3), stop_at=(3, 3))
                r16 = work.tile([128, 4 * b], F16, tag="r16")
                nc.scalar.activation(r16[:], r_ps[:], sig)

                bias_mm(hn_ps[:], "hn")
                mm_k(hn_ps, 1, (0, 1, 2, 3), stop_at=(3, 3))
                rhn16 = work.tile([128, 4 * b], F16, tag="rhn")
                nc.vector.tensor_mul(rhn16[:], r16[:], hn_ps[:])

                if not first:
                    mm_k(in_ps, 2, (2, 3))
                id_mm = chain(nc.tensor.matmul(in_ps[:], ident_sb[:],
                                               rhn16[:], start=False,
                                               stop=True,
                                               skip_group_check=True))
                n16 = work.tile([128, 4 * b], F16, tag="n")
                tanh_inst = nc.scalar.activation(n16[:], in_ps[:], tanh)
                d16 = work.tile([128, 4 * b], F16, tag="d")
                nc.vector.tensor_sub(d16[:], h16[:], n16[:])

                # z blocks last; split into two PSUM banks so the tail is
                # produced in two halves (early release of k0/k1 for t+1)
                bias_mm(za_ps[:], "z", cols=slice(0, 2 * b))
                bias_mm(zb_ps[:], "z", cols=slice(2 * b, 4 * b))
                mm_k(za_ps, zg, (0, 1, 2, 3), tiles=(0, 1), col_of=(0, 1),
                     stop_at=(3, 1))
                mm_k(zb_ps, zg, (0, 1, 2, 3), tiles=(2, 3), col_of=(0, 1),
                     stop_at=(3, 3))
                za16 = work.tile([128, 2 * b], F16, tag="za16")
                siga = nc.scalar.activation(za16[:], za_ps[:], sig)
                add_dep_helper(siga.ins, tanh_inst.ins, sync=False,
                               reason="sigma_za waits on tanh (ACT order)")
                zb16 = work.tile([128, 2 * b], F16, tag="zb16")
                sigb = nc.scalar.activation(zb16[:], zb_ps[:], sig)
                add_dep_helper(sigb.ins, siga.ins, sync=False,
                               reason="sigma_zb waits on sigma_za (ACT order)")

                h16_new = state.tile([128, 4 * b], F16, tag="h16")
                zda = work.tile([128, 2 * b], F16, tag="zda")
                nc.vector.tensor_mul(zda[:], za16[:], d16[:, 0 : 2 * b])
                ha = nc.vector.tensor_add(h16_new[:, 0 : 2 * b], zda[:],
                                          n16[:, 0 : 2 * b])
                zdb = work.tile([128, 2 * b], F16, tag="zdb")
                zdb_mm = nc.vector.tensor_mul(zdb[:], zb16[:],
                                              d16[:, 2 * b : 4 * b])
                add_dep_helper(zdb_mm.ins, ha.ins, sync=False,
                               reason="zdb after h'_a (DVE order, early k0/k1)")
                nc.vector.tensor_add(h16_new[:, 2 * b : 4 * b], zdb[:],
                                     n16[:, 2 * b : 4 * b])
                nc.sync.dma_start(out_d[t], h16_new[:])
                h16 = h16_new

    if not nc.is_finalized():
        nc.finalize()
    return nc


def _prep_host(encoder_outputs, W_ih, W_hh, b_ih, b_hh, T, n_cores, b):
    """Shard + lay out host inputs; returns per-core in_maps."""
    W_ih = np.asarray(W_ih, dtype=np.float32)
    W_hh = np.asarray(W_hh, dtype=np.float32)
    b_ih = np.asarray(b_ih, dtype=np.float32)
    b_hh = np.asarray(b_hh, dtype=np.float32)
    enc = np.asarray(encoder_outputs, dtype=np.float32)

    W_r = W_ih[:H] + W_hh[:H]
    W_z = W_ih[H : 2 * H] + W_hh[H : 2 * H]
    W_hn = W_hh[2 * H :]
    W_in = W_ih[2 * H :]

    def blocks_of(*gates):
        cols = []
        for Wm in gates:
            WmT = np.ascontiguousarray(Wm.T)  # (512, 512)
            for tt in range(4):
                for k in range(KT):
                    cols.append(
                        WmT[128 * k : 128 * (k + 1), 128 * tt : 128 * (tt + 1)]
                    )
        return np.concatenate(cols, axis=1).astype(np.float16)

    wc_host = blocks_of(W_r, W_hn, W_in, W_z)            # (128, 64*128)
    w1_host = blocks_of(W_hh[:H], W_hh[2 * H :], W_hh[H : 2 * H])

    def bias128(bvec):
        m = np.zeros((128, 128), np.float16)
        m[0:4, :] = bvec.reshape(4, 128).astype(np.float16)
        return m

    bst = np.concatenate([
        bias128(b_ih[:H] + b_hh[:H]),
        bias128(b_hh[2 * H :]),
        bias128(b_ih[2 * H :]),
        bias128(b_ih[H : 2 * H] + b_hh[H : 2 * H]),
    ], axis=1)  # (128, 512)
    ones = np.zeros((128, 4 * b), np.float16)
    ones[0:4] = np.kron(np.eye(4, dtype=np.float16),
                        np.ones((1, b), np.float16))
    ident = np.eye(128, dtype=np.float16)

    h0 = enc[0, :, -1, :]  # (128, 512)
    in_maps = []
    for c in range(n_cores):
        h0c = h0[c * b : (c + 1) * b]  # (b, 512)
        h0t = np.ascontiguousarray(
            h0c.reshape(b, KT, 128).transpose(2, 1, 0).reshape(128, KT * b)
        ).astype(np.float16)
        in_maps.append({
            "wc": wc_host, "w1": w1_host, "bst": bst,
            "ones": ones, "ident": ident, "h0t": h0t,
        })
    return in_maps


def _gather(results, T, n_cores, b):
    out = np.empty((T, BATCH, H), dtype=np.float32)
    for c in range(n_cores):
        oc = results[c]["outT"]  # (T, 128, KT*b) fp16, free = [k][j]
        out[:, c * b : (c + 1) * b, :] = (
            oc.reshape(T, 128, KT, b).transpose(0, 3, 2, 1).reshape(T, b, H)
            .astype(np.float32)
        )
    return out


_CACHE = {}


def kernel(encoder_outputs, W_ih, W_hh, b_ih, b_hh, out_len):
    T = int(out_len)
    assert T == T_STEPS, f"built for T={T_STEPS}, got {T}"
    key = (T, N_CORES)
    if key not in _CACHE:
        _CACHE[key] = _build(T, B_LOC)
    nc = _CACHE[key]

    in_maps = _prep_host(encoder_outputs, W_ih, W_hh, b_ih, b_hh,
                         T, N_CORES, B_LOC)
    res = run_bass_kernel_spmd(nc, in_maps, core_ids=list(range(N_CORES)))
    out = _gather(res.results, T, N_CORES, B_LOC)
    return out.reshape(T * BATCH, 1, H)
